# revision 1
# baseline (speedup 1.0000x reference)
"""GroupFC kernel for Trainium2, data-parallel across 8 NeuronCores.

Problem: out = data @ W.T + b
  data: [32768, 1024] f32, W: [1024, 1024] f32 (block-diagonal-masked), b: [1024] f32

Strategy:
  - Shard batch dim across 8 cores (4096 rows each); replicate W, b.
  - Host-side: cast data shard + W to bf16, pre-transpose so the contraction
    dim (in_features) lands on SBUF partitions; broadcast b to [128, 1024].
  - On-chip per core: the whole 8 MiB bf16 data shard is SBUF-resident as
    64 independent [128, 512] chunk tiles (fine-grained deps let the PE
    start as soon as the first chunks land). out_tile[128b, 512o] is
    accumulated over 8 K-tiles in PSUM (bf16 operands, fp32 accumulate),
    bias is added during PSUM->SBUF evacuation on DVE, stores go out in
    natural [batch, out] layout.
"""

import os
import sys
from contextlib import ExitStack

import numpy as np

try:
    import concourse.bass as bass  # noqa: F401
except ImportError:
    sys.path.insert(0, "/opt/trn_rl_repo")

import ml_dtypes

import concourse.tile as tile
from concourse import bacc, mybir
from concourse.bass_utils import run_bass_kernel_spmd

N_CORES = 8
BATCH = 32768
SHARD = BATCH // N_CORES  # 4096
IN_DIM = 1024
OUT_DIM = 1024
P = 128
KT = IN_DIM // P  # 8 contraction tiles
NFREE = 512  # psum bank free-dim (fp32)
CCHUNK = 1024  # batch columns per data chunk tile
NCHUNKS = SHARD // CCHUNK  # 4
SUBS_PER_CHUNK = CCHUNK // P  # 8

_CACHE = {}


def _build():
    nc = bacc.Bacc("TRN2", target_bir_lowering=False, debug=False)
    dT = nc.dram_tensor(
        "dT", [IN_DIM, SHARD], mybir.dt.bfloat16, kind="ExternalInput"
    ).ap()
    wT = nc.dram_tensor(
        "wT", [IN_DIM, OUT_DIM], mybir.dt.bfloat16, kind="ExternalInput"
    ).ap()
    biasb = nc.dram_tensor(
        "biasb", [P, OUT_DIM], mybir.dt.float32, kind="ExternalInput"
    ).ap()
    out = nc.dram_tensor(
        "out", [SHARD, OUT_DIM], mybir.dt.float32, kind="ExternalOutput"
    ).ap()

    with tile.TileContext(nc) as tc:
        with ExitStack() as ctx:
            wp = ctx.enter_context(tc.tile_pool(name="w", bufs=1))
            bp = ctx.enter_context(tc.tile_pool(name="bias", bufs=1))
            dp = ctx.enter_context(tc.tile_pool(name="d", bufs=1))
            pp = ctx.enter_context(tc.tile_pool(name="psum", bufs=4, space="PSUM"))
            op = ctx.enter_context(tc.tile_pool(name="o", bufs=6))

            # w_tiles[k][nh]: [128, 512] halves of wT k-tile.
            w_tiles = [[None] * 2 for _ in range(KT)]
            # d0a/d0b: first chunk split as two [128, 512] tiles (subs 0-3 /
            # 4-7); d_tiles[k][c] for c>=1: [128, 1024] chunks (8 subs each).
            d0 = [[None] * 2 for _ in range(KT)]
            d_tiles = [[None] * NCHUNKS for _ in range(KT)]

            # Load plan: small primer transfers first, in the exact order the
            # k-major ramp consumes them, alternated across two load queues.
            loads = [("w", 0, 0), ("d0", 0, 0), ("w", 0, 1)]
            for k in range(1, KT):
                loads.append(("w", k, 0))
                loads.append(("w", k, 1))
                loads.append(("d0", k, 0))
            loads.append(("bias", 0, 0))
            for k in range(KT):
                loads.append(("d0", k, 1))
            for c in range(1, NCHUNKS):
                for k in range(KT):
                    loads.append(("d", k, c))

            bias_t = None
            for i, (kind, k, j) in enumerate(loads):
                eng = nc.scalar if i % 2 == 0 else nc.sync
                if kind == "w":
                    wt = wp.tile([P, NFREE], mybir.dt.bfloat16, tag=f"w{k}_{j}")
                    eng.dma_start(
                        out=wt[:],
                        in_=wT[k * P : (k + 1) * P, j * NFREE : (j + 1) * NFREE],
                    )
                    w_tiles[k][j] = wt
                elif kind == "bias":
                    bias_t = bp.tile([P, OUT_DIM], mybir.dt.float32)
                    eng.dma_start(out=bias_t[:], in_=biasb[:, :])
                elif kind == "d0":
                    dt_t = dp.tile([P, NFREE], mybir.dt.bfloat16, tag=f"d0_{k}_{j}")
                    eng.dma_start(
                        out=dt_t[:],
                        in_=dT[k * P : (k + 1) * P, j * NFREE : (j + 1) * NFREE],
                    )
                    d0[k][j] = dt_t
                else:
                    dt_t = dp.tile([P, CCHUNK], mybir.dt.bfloat16, tag=f"d{k}_{j}")
                    eng.dma_start(
                        out=dt_t[:],
                        in_=dT[k * P : (k + 1) * P, j * CCHUNK : (j + 1) * CCHUNK],
                    )
                    d_tiles[k][j] = dt_t

            def sub_lhsT(k, sub):
                if sub < 4:
                    return d0[k][0][:, sub * P : (sub + 1) * P]
                if sub < 8:
                    return d0[k][1][:, (sub - 4) * P : (sub - 3) * P]
                c = sub // SUBS_PER_CHUNK
                s = sub - c * SUBS_PER_CHUNK
                return d_tiles[k][c][:, s * P : (s + 1) * P]

            def evacuate(sub, ps0, ps1):
                ot = op.tile([P, OUT_DIM], mybir.dt.float32, tag="ot")
                nc.vector.tensor_add(ot[:, 0:NFREE], ps0[:], bias_t[:, 0:NFREE])
                nc.vector.tensor_add(
                    ot[:, NFREE:OUT_DIM], ps1[:], bias_t[:, NFREE:OUT_DIM]
                )
                r0 = sub * P
                # Early stores go on gpsimd (software DGE: slow, but their
                # completion is latency-insensitive mid-kernel). From sub 8 on
                # the HWDGE load queues are drained, so stores go there as
                # halves, alternating, keeping the end-of-kernel drain to one
                # 256 KiB transfer per HW queue.
                if sub >= 8:
                    e0 = nc.scalar if sub % 2 == 0 else nc.sync
                    e1 = nc.sync if sub % 2 == 0 else nc.scalar
                    e0.dma_start(out=out[r0 : r0 + P, 0:NFREE], in_=ot[:, 0:NFREE])
                    e1.dma_start(
                        out=out[r0 : r0 + P, NFREE:OUT_DIM], in_=ot[:, NFREE:OUT_DIM]
                    )
                else:
                    nc.gpsimd.dma_start(out=out[r0 : r0 + P, :], in_=ot[:])

            # PE pre-warm: the PE is DMA-idle for the first ~10 us, so its
            # HAM clock gate holds it at 1.2 GHz for the first ~3.4 us of
            # real work. Run dummy matmuls on a zeroed scratch tile into the
            # first ramp bank while loads stream in, so the clock is at
            # 2.4 GHz when the real accumulation chain starts.
            scratch = wp.tile([P, NFREE], mybir.dt.bfloat16, tag="warm_scratch")
            nc.vector.memset(scratch[:], 0)

            # Ramp: k-major over the first 4 subtiles (8 PSUM banks live) so
            # each arriving (w[k], d0a[k]) pair unlocks 8 matmuls.
            ramp = [
                (pp.tile([P, NFREE], mybir.dt.float32, tag="ps0", name=f"rps0_{s}"),
                 pp.tile([P, NFREE], mybir.dt.float32, tag="ps1", name=f"rps1_{s}"))
                for s in range(4)
            ]
            for wi in range(10):
                nc.tensor.matmul(
                    ramp[0][0][:], scratch[:, 0:P], scratch[:],
                    start=True, stop=True,
                )
            for k in range(KT):
                for s in range(4):
                    lhsT = sub_lhsT(k, s)
                    nc.tensor.matmul(
                        ramp[s][0][:], lhsT, w_tiles[k][0][:],
                        start=(k == 0), stop=(k == KT - 1),
                    )
                    nc.tensor.matmul(
                        ramp[s][1][:], lhsT, w_tiles[k][1][:],
                        start=(k == 0), stop=(k == KT - 1),
                    )
            for s in range(4):
                evacuate(s, ramp[s][0], ramp[s][1])

            # Steady state: sub-major.
            for sub in range(4, SHARD // P):
                ps0 = pp.tile([P, NFREE], mybir.dt.float32, tag="ps0")
                ps1 = pp.tile([P, NFREE], mybir.dt.float32, tag="ps1")
                for k in range(KT):
                    lhsT = sub_lhsT(k, sub)
                    nc.tensor.matmul(
                        ps0[:], lhsT, w_tiles[k][0][:],
                        start=(k == 0), stop=(k == KT - 1),
                    )
                    nc.tensor.matmul(
                        ps1[:], lhsT, w_tiles[k][1][:],
                        start=(k == 0), stop=(k == KT - 1),
                    )
                evacuate(sub, ps0, ps1)

    nc.compile()
    return nc


def _get_nc():
    if "nc" not in _CACHE:
        _CACHE["nc"] = _build()
    return _CACHE["nc"]


def _prep_inputs(data, W, b):
    data = np.asarray(data, dtype=np.float32)
    W = np.asarray(W, dtype=np.float32)
    b = np.asarray(b, dtype=np.float32)
    wT = np.ascontiguousarray(W.astype(ml_dtypes.bfloat16).T)  # [in, out] bf16
    bias_bc = np.ascontiguousarray(
        np.broadcast_to(b[None, :], (P, OUT_DIM))
    )  # [128, 1024] f32
    in_maps = []
    for c in range(N_CORES):
        shard = data[c * SHARD : (c + 1) * SHARD]  # [4096, 1024] f32
        dT = np.ascontiguousarray(shard.astype(ml_dtypes.bfloat16).T)  # [in, batch]
        in_maps.append({"dT": dT, "wT": wT, "biasb": bias_bc})
    return in_maps


def _run(data, W, b, trace=False, **trace_kw):
    nc = _get_nc()
    in_maps = _prep_inputs(data, W, b)
    res = run_bass_kernel_spmd(nc, in_maps, list(range(N_CORES)), trace=trace, **trace_kw)
    out = np.concatenate(
        [np.asarray(res.results[c]["out"], dtype=np.float32) for c in range(N_CORES)],
        axis=0,
    )
    return out, res


def kernel(**inputs) -> np.ndarray:
    out, _ = _run(inputs["data"], inputs["W"], inputs["b"])
    return out

